# revision 12
# baseline (speedup 1.0000x reference)
"""HB-LSTM cell fused Trainium2 kernel, data-parallel over 8 NeuronCores.

Computes, for gate order (f, i, o, u, k):
    pre  = x @ Wx[g].T + bx[g] + h_prev @ Uh[g].T + bh[g]
    f,i,o,u = sigmoid(pre[0..3]);  c = tanh(pre[4])
    kp = u*c + (1-u)*kp_prev
    k  = f*k_prev + i*kp
    h  = o*tanh(k)
Returns (h, k, kp), each [B, H] float32.

Sharding: batch dim B=65536 split across 8 cores (8192 rows each); weight
stacks replicated to every core.

Per-core structure (ramp group of 4 b-tiles, 7 groups of 8, drain group
of 4; b-tile = 128 batch rows):
  - Weight preamble: per i-chunk staged SWDGE load (cast f32->bf16) +
    one whole-tile xbar transpose per chunk yields all matmul rhs tiles;
    tanh-gate (g=4) weights/bias pre-scaled by 2 so one sigmoid covers
    all 1280 gate cols (tanh(x) = 2*sigmoid(2x) - 1).
  - Ramp group loads x/h f32 via HWDGE on the scalar ring (parallel to
    the SWDGE weight loads) + DVE casts, so the first matmuls start
    ~14us in; steady-state groups use SWDGE cast-in-DMA loads + one
    whole-group xbar transpose (feature-major lhsT tiles).
  - Per b-tile: 5-gate pre-activations accumulate in one [128,1280] PSUM
    tile: bias via K=1 ones-matmul (start), then 12 bf16 matmuls, then
    one sigmoid on ACT into fp16 gates.
  - Elementwise tail entirely in fp16 at group granularity; k_prev /
    kp_prev cast to fp16 in the load DMA; outputs stored as fp16
    (upcast to f32 on host).
"""

import numpy as np

import concourse.bacc as bacc
import concourse.mybir as mybir
from concourse import tile
from concourse.bass_utils import run_bass_kernel_spmd

N_CORES = 8
B = 65536
IN = 256
H = 256
G5 = 5
BL = B // N_CORES          # rows per core
NT = BL // 128             # 64 b-tiles per core
GROUP = 8                  # b-tiles per steady-state group
RAMP = 4                   # b-tiles in the first/last group
DG = G5 * H                # 1280 = all-gate column span
F32 = mybir.dt.float32
BF16 = mybir.dt.bfloat16
F16 = mybir.dt.float16
GDT = BF16                 # GEMM compute dtype
DT = F16                   # elementwise-tail dtype
AF = mybir.ActivationFunctionType
ALU = mybir.AluOpType

PSUM_BUFS = 2

_CACHE = {}


def _build():
    if "nc" in _CACHE:
        return _CACHE["nc"]

    nc = bacc.Bacc("TRN2", target_bir_lowering=False, debug=False,
                   num_devices=N_CORES)

    x_d = nc.dram_tensor("x", [BL, IN], F32, kind="ExternalInput")
    h_d = nc.dram_tensor("h_prev", [BL, H], F32, kind="ExternalInput")
    k_d = nc.dram_tensor("k_prev", [BL, H], F32, kind="ExternalInput")
    kp_d = nc.dram_tensor("kp_prev", [BL, H], F32, kind="ExternalInput")
    wx_d = nc.dram_tensor("Wx", [G5, H, IN], F32, kind="ExternalInput")
    bx_d = nc.dram_tensor("bx", [G5, H], F32, kind="ExternalInput")
    uh_d = nc.dram_tensor("Uh", [G5, H, H], F32, kind="ExternalInput")
    bh_d = nc.dram_tensor("bh", [G5, H], F32, kind="ExternalInput")
    ho_d = nc.dram_tensor("h_out", [BL, H], DT, kind="ExternalOutput")
    ko_d = nc.dram_tensor("k_out", [BL, H], DT, kind="ExternalOutput")
    kpo_d = nc.dram_tensor("kp_out", [BL, H], DT, kind="ExternalOutput")

    groups = [(0, RAMP)]
    s = RAMP
    while s + GROUP <= NT - RAMP:
        groups.append((s, GROUP))
        s += GROUP
    groups.append((s, NT - s))

    with tile.TileContext(nc) as tc:
        with tc.tile_pool(name="const", bufs=1) as cpool:
            # WTt[c][ip, side, g, hc, p] bf16: transposed weight stacks,
            # i-chunk c on partitions. Views WT[side][c] = [128, 1280] rhs.
            WTt = [cpool.tile([128, 2, G5, 2, 128], GDT, name=f"WT{c}",
                              tag=f"WT{c}") for c in range(2)]
            WT = {(s_, c): WTt[c][:, s_].rearrange("p g hc i -> p (g hc i)")
                  for s_ in range(2) for c in range(2)}
            bs16 = cpool.tile([1, DG], GDT, tag="bs16")
            ones16 = cpool.tile([1, 128], GDT, tag="ones16")

            x_cm = x_d.ap().rearrange("(n p) (c q) -> p n c q", p=128, q=128)
            h_cm = h_d.ap().rearrange("(n p) (c q) -> p n c q", p=128, q=128)
            k_t = k_d.ap().rearrange("(n p) i -> p n i", p=128)
            kp_t = kp_d.ap().rearrange("(n p) i -> p n i", p=128)
            ho_t = ho_d.ap().rearrange("(n p) i -> p n i", p=128)
            ko_t = ko_d.ap().rearrange("(n p) i -> p n i", p=128)
            kpo_t = kpo_d.ap().rearrange("(n p) i -> p n i", p=128)

            with tc.tile_pool(name="io", bufs=2) as io, \
                 tc.tile_pool(name="work", bufs=2) as work, \
                 tc.tile_pool(name="psum", bufs=PSUM_BUFS, space="PSUM") as pp, \
                 tc.tile_pool(name="wload", bufs=1) as wload:
                # -- 1. weight SWDGE loads (cast f32->bf16), c-chunk split
                stc = [wload.tile([128, 2, G5, 2, 128], GDT, name=f"stc{c}",
                                  tag=f"stc{c}")
                       for c in range(2)]
                wsrc = [w.ap().rearrange("g (hc p) (c i) -> p c g hc i",
                                         p=128, i=128)
                        for w in (wx_d, uh_d)]
                for c in range(2):
                    for s_ in range(2):
                        nc.gpsimd.dma_start(stc[c][:, s_], wsrc[s_][:, c])

                # -- 2. ramp group x/h via HWDGE f32 (scalar ring) + casts
                xh16_0 = io.tile([128, 2, RAMP, 2, 128], GDT, tag="xh16r")
                x32 = wload.tile([128, RAMP, 2, 128], F32, tag="x32")
                h32 = wload.tile([128, RAMP, 2, 128], F32, tag="h32")
                nc.scalar.dma_start(x32[:], x_cm[:, 0:RAMP])
                nc.scalar.dma_start(h32[:], h_cm[:, 0:RAMP])
                nc.vector.tensor_copy(xh16_0[:, 0], x32[:])
                nc.vector.tensor_copy(xh16_0[:, 1], h32[:])
                xhT_0 = work.tile([128, 2, RAMP, 2, 128], GDT, tag="xhTr")
                nc.scalar.dma_start(xhT_0[:], xh16_0[:], transpose=True)

                # -- 3. weight scales (tanh gate x2) + transposes
                for c in range(2):
                    nc.vector.tensor_scalar_mul(stc[c][:, :, 4],
                                                stc[c][:, :, 4], 2.0)
                    nc.sync.dma_start(WTt[c][:], stc[c][:], transpose=True)

                # -- 4. bias row: bs16 = bx + bh, tanh gate x2, cast bf16
                bxr = wload.tile([1, DG], F32, tag="bxr")
                bhr = wload.tile([1, DG], F32, tag="bhr")
                nc.gpsimd.dma_start(
                    bxr[:], bx_d.ap().rearrange("g h -> (g h)").unsqueeze(0))
                nc.gpsimd.dma_start(
                    bhr[:], bh_d.ap().rearrange("g h -> (g h)").unsqueeze(0))
                bsr = wload.tile([1, DG], F32, tag="bsr")
                nc.vector.tensor_add(bsr[:], bxr[:], bhr[:])
                nc.vector.tensor_scalar_mul(bsr[:, 4 * H:], bsr[:, 4 * H:],
                                            2.0)
                nc.vector.tensor_copy(bs16[:], bsr[:])
                nc.vector.memset(ones16[:], 1.0)

                # -- 5. ramp group k/kp (SWDGE cast f32->fp16)
                kr_0 = io.tile([128, GROUP, H], DT, tag="kr")
                kpp_0 = io.tile([128, GROUP, H], DT, tag="kpp")
                nc.gpsimd.dma_start(kr_0[:, :RAMP], k_t[:, 0:RAMP, :])
                nc.gpsimd.dma_start(kpp_0[:, :RAMP], kp_t[:, 0:RAMP, :])

                # -- 6. main loop
                for s0, sz in groups:
                    nsl = slice(s0, s0 + sz)
                    if s0 == 0:
                        xhT, kr, kpp = xhT_0, kr_0, kpp_0
                    else:
                        xh16 = io.tile([128, 2, sz, 2, 128], GDT,
                                       name=f"xh16_{s0}",
                                       tag="xh16" if sz == GROUP
                                       else "xh16r")
                        nc.gpsimd.dma_start(xh16[:, 0], x_cm[:, nsl])
                        nc.gpsimd.dma_start(xh16[:, 1], h_cm[:, nsl])
                        kr = io.tile([128, GROUP, H], DT, tag="kr")
                        kpp = io.tile([128, GROUP, H], DT, tag="kpp")
                        nc.gpsimd.dma_start(kr[:, :sz], k_t[:, nsl, :])
                        nc.gpsimd.dma_start(kpp[:, :sz], kp_t[:, nsl, :])
                        xhT = work.tile([128, 2, sz, 2, 128], GDT,
                                        name=f"xhT_{s0}",
                                        tag="xhT" if sz == GROUP
                                        else "xhTr")
                        nc.sync.dma_start(xhT[:], xh16[:], transpose=True)
                    kp_o = io.tile([128, GROUP, H], DT, tag="kp_o")
                    k_o = io.tile([128, GROUP, H], DT, tag="k_o")
                    h_o = io.tile([128, GROUP, H], DT, tag="h_o")
                    gates = work.tile([128, GROUP, DG], DT, tag="gates")

                    for j in range(sz):
                        ps = pp.tile([128, DG], F32, tag="ps")
                        for n0 in range(0, DG, 512):
                            n1 = min(n0 + 512, DG)
                            nc.tensor.matmul(ps[:, n0:n1], ones16[:],
                                             bs16[:, n0:n1],
                                             start=True, stop=False)
                        for si in range(2):
                            for c in range(2):
                                lhsT = xhT[:, si, j, c]
                                last = si == 1 and c == 1
                                for n0 in range(0, DG, 512):
                                    n1 = min(n0 + 512, DG)
                                    nc.tensor.matmul(
                                        ps[:, n0:n1], lhsT,
                                        WT[si, c][:, n0:n1],
                                        start=False, stop=last)
                        # all 5 gates in one sigmoid (tanh gate pre-scaled)
                        nc.scalar.activation(gates[:, j, :], ps[:], AF.Sigmoid)

                    # ---- group elementwise tail, fp16, N = sz*256 ----
                    f_ = gates[:, :sz, 0:256]
                    i_ = gates[:, :sz, 256:512]
                    o_ = gates[:, :sz, 512:768]
                    u_ = gates[:, :sz, 768:1024]
                    s4 = gates[:, :sz, 1024:1280]
                    c2 = work.tile([128, GROUP, H], DT, tag="c2")
                    nc.vector.tensor_scalar(c2[:, :sz], s4, 2.0, -1.0,
                                            ALU.mult, ALU.add)
                    d = work.tile([128, GROUP, H], DT, tag="d")
                    nc.vector.tensor_sub(d[:, :sz], c2[:, :sz], kpp[:, :sz])
                    e = work.tile([128, GROUP, H], DT, tag="e")
                    nc.vector.tensor_mul(e[:, :sz], u_, d[:, :sz])
                    nc.vector.tensor_add(kp_o[:, :sz], e[:, :sz], kpp[:, :sz])
                    m = work.tile([128, GROUP, H], DT, tag="d")
                    nc.vector.tensor_mul(m[:, :sz], f_, kr[:, :sz])
                    n = work.tile([128, GROUP, H], DT, tag="e")
                    nc.vector.tensor_mul(n[:, :sz], i_, kp_o[:, :sz])
                    nc.vector.tensor_add(k_o[:, :sz], m[:, :sz], n[:, :sz])
                    tk = work.tile([128, GROUP, H], DT, tag="c2")
                    nc.scalar.activation(tk[:, :sz], k_o[:, :sz], AF.Tanh)
                    nc.vector.tensor_mul(h_o[:, :sz], o_, tk[:, :sz])

                    nc.scalar.dma_start(kpo_t[:, nsl, :], kp_o[:, :sz])
                    nc.scalar.dma_start(ko_t[:, nsl, :], k_o[:, :sz])
                    nc.scalar.dma_start(ho_t[:, nsl, :], h_o[:, :sz])

    nc.compile()
    _CACHE["nc"] = nc
    return nc


def make_in_maps(np_inputs):
    x = np.asarray(np_inputs["x"], dtype=np.float32)
    h_prev = np.asarray(np_inputs["h_prev"], dtype=np.float32)
    k_prev = np.asarray(np_inputs["k_prev"], dtype=np.float32)
    kp_prev = np.asarray(np_inputs["kp_prev"], dtype=np.float32)
    Wx = np.ascontiguousarray(np.asarray(np_inputs["Wx"], dtype=np.float32))
    bx = np.ascontiguousarray(np.asarray(np_inputs["bx"], dtype=np.float32))
    Uh = np.ascontiguousarray(np.asarray(np_inputs["Uh"], dtype=np.float32))
    bh = np.ascontiguousarray(np.asarray(np_inputs["bh"], dtype=np.float32))
    in_maps = []
    for c in range(N_CORES):
        sl = slice(c * BL, (c + 1) * BL)
        in_maps.append({
            "x": np.ascontiguousarray(x[sl]),
            "h_prev": np.ascontiguousarray(h_prev[sl]),
            "k_prev": np.ascontiguousarray(k_prev[sl]),
            "kp_prev": np.ascontiguousarray(kp_prev[sl]),
            "Wx": Wx, "bx": bx, "Uh": Uh, "bh": bh,
        })
    return in_maps


def kernel(x, h_prev, k_prev, kp_prev, Wx, bx, Uh, bh):
    nc = _build()
    in_maps = make_in_maps(dict(x=x, h_prev=h_prev, k_prev=k_prev,
                                kp_prev=kp_prev, Wx=Wx, bx=bx, Uh=Uh, bh=bh))
    res = run_bass_kernel_spmd(nc, in_maps, list(range(N_CORES)))
    h_out = np.concatenate(
        [np.asarray(res.results[c]["h_out"]).astype(np.float32)
         for c in range(N_CORES)], axis=0)
    k_out = np.concatenate(
        [np.asarray(res.results[c]["k_out"]).astype(np.float32)
         for c in range(N_CORES)], axis=0)
    kp_out = np.concatenate(
        [np.asarray(res.results[c]["kp_out"]).astype(np.float32)
         for c in range(N_CORES)], axis=0)
    return (h_out, k_out, kp_out)
